# revision 1
# baseline (speedup 1.0000x reference)
"""Trainium2 Bass kernel for the DNM dendritic linear layer.

Reference math (K=0.5, QS=0.1):
    syn[b,o,m,i] = relu(K*(x[b,i]*W[o,m,i] - q[o,m,i]))
    dend[b,o,m]  = relu(sum_i syn)
    soma[b,o]    = sum_m dend
    out[b,o]     = relu(K*(soma - QS))

Key identity (W >= 0 a.s., W ~ U[0,1)):
    relu(K*(x*W - q)) = (K*W) * relu(x - q/W)
so with Wh = K*W and V = q/W:
    dend_pre[b,om] = sum_i Wh[om,i] * relu(x[b,i] - V[om,i])

Device strategy (per core, tensor-parallel over OUT: 16 of 128 rows/core,
om = o*8+m gives OM=128 (o,m) pairs per core):
  - x transposed on host: xT[i, b] (fp16), i on partitions (4 chunks of 128).
  - u'[om,c] = relu(xT_chunk_c - V[om, chunk_c]) -- a per-partition-scalar
    op, split between DVE tensor_scalar((x + (-V)) max 0) and ACT
    activation(Relu, bias=-V); output fp16 [128i x 512b].
  - weighted i-sum on PE: matmul with a masked stationary [128 x 32]
    holding Wh[om, chunk] in column om%32 (zeros elsewhere), accumulating
    into PSUM rows [32*(om//32) .. +32)  (output base partitions must be
    32-aligned).  Matmuls are interleaved across the four 32-col groups
    so the PE overlaps them (col-tiling concurrency).
  - epilogue: dend = relu(PSUM) on ACT -> m-sum via one fp32 matmul with
    a 0/1 stationary [128 x 16] -> out = relu(K*soma - K*QS) -> DMA.

All W/q-derived constants (masked stationaries, -V, m-sum matrix) are
packed on the host inside kernel() and shipped as extra inputs; the
device does all x-dependent compute.
"""

import numpy as np

B, OUT, MDIM, IN = 512, 128, 8, 512
NCORES = 8
OLOC = OUT // NCORES          # 16 output rows per core
OM = OLOC * MDIM              # 128 (o,m) pairs per core
NCH = IN // 128               # 4 i-chunks
KCONST, QS = 0.5, 0.1
STATW = 132                   # per-om stride in the masked stationary buffer
NGRP = 8                      # statw DMA split granularity (16 oms each)
ACT_MOD = 4                   # every ACT_MOD-th (om,c) unit runs on ACT engine

_CACHE = {}


def _build():
    import concourse.bacc as bacc
    import concourse.tile as tile
    from concourse.mybir import AluOpType as alu, ActivationFunctionType as actf, dt

    nc = bacc.Bacc("TRN2", target_bir_lowering=False, debug=False)
    xT_d = nc.dram_tensor("xT", [IN, B], dt.float16, kind="ExternalInput").ap()
    negV_d = nc.dram_tensor("negV", [128, NCH * OM], dt.float32, kind="ExternalInput").ap()
    WhT_d = nc.dram_tensor("WhT", [128, NCH * OM], dt.float16, kind="ExternalInput").ap()
    msum_d = nc.dram_tensor("msum", [128, OLOC], dt.float32, kind="ExternalInput").ap()
    out_d = nc.dram_tensor("out", [OLOC, B], dt.float32, kind="ExternalOutput").ap()

    with tile.TileContext(nc) as tc:
        with tc.tile_pool(name="const", bufs=1) as cpool, \
             tc.tile_pool(name="upool", bufs=12) as upool, \
             tc.tile_pool(name="ppool", bufs=1, space="PSUM") as ppool:

            # Input DMAs spread across the two HWDGE issuers (SP + ACT) and
            # gpsimd SWDGE, ordered by first use.  Only ~1MB of input total:
            # the masked stationary buffer is built on device from WhT.
            xT_sb = cpool.tile([128, NCH * B], dt.float16)
            negV = cpool.tile([128, NCH * OM], dt.float32)
            WhT = cpool.tile([128, NCH * OM], dt.float16)
            msum = cpool.tile([128, OLOC], dt.float32)

            nc.sync.dma_start(negV[:], negV_d[:, :])
            nc.scalar.dma_start(xT_sb[:, 0 * B:1 * B], xT_d[0 * 128:1 * 128, :])
            nc.sync.dma_start(WhT[:], WhT_d[:, :])
            nc.scalar.dma_start(xT_sb[:, 1 * B:2 * B], xT_d[1 * 128:2 * 128, :])
            nc.sync.dma_start(xT_sb[:, 2 * B:3 * B], xT_d[2 * 128:3 * 128, :])
            nc.scalar.dma_start(xT_sb[:, 3 * B:4 * B], xT_d[3 * 128:4 * 128, :])
            nc.gpsimd.dma_start(msum[:], msum_d[:, :])

            # Masked stationaries: zeros except Wh col of (om,c) at flat
            # om*STATW + 33c.  Zeroing split DVE/ACT (runs under the fixed
            # preamble + DMA window), then 4 strided scatter copies.
            stat = cpool.tile([128, OM * STATW], dt.float16)
            stat_u32 = stat.bitcast(dt.uint32)
            half = (OM * STATW) // 4  # u32 elems per half
            nc.vector.memset(stat_u32[:, :half], 0)
            nc.scalar.memzero(stat[:, OM * STATW // 2:])
            stat3 = stat.rearrange("p (om k) -> p om k", k=STATW)
            for c in range(NCH):
                src3 = WhT[:, c * OM:(c + 1) * OM].rearrange("p (a b) -> p a b", b=1)
                nc.vector.tensor_copy(stat3[:, :, 33 * c:33 * c + 1], src3)

            psum_acc = ppool.tile([128, B], dt.float32, tag="acc")

            idx = 0
            for j in range(32):
                for c in range(NCH):
                    for g in range(4):
                        om = g * 32 + j
                        u = upool.tile([128, B], dt.float16, tag="u")
                        col = c * OM + om
                        if idx % ACT_MOD == ACT_MOD - 1:
                            nc.scalar.activation(u[:], xT_sb[:, c * B:(c + 1) * B],
                                                 actf.Relu,
                                                 bias=negV[:, col:col + 1],
                                                 scale=1.0)
                        else:
                            nc.vector.tensor_scalar(u[:], xT_sb[:, c * B:(c + 1) * B],
                                                    negV[:, col:col + 1], 0.0,
                                                    alu.add, alu.max)
                        off = om * STATW + 33 * c - j
                        nc.tensor.matmul(psum_acc[g * 32:(g + 1) * 32, :],
                                         stat[:, off:off + 32], u[:],
                                         start=(j == 0 and c == 0),
                                         stop=(j == 31 and c == NCH - 1),
                                         tile_position=(0, g * 32))
                        idx += 1

            # dend = relu(psum) (fp32) on ACT, then soma[o,b] = sum_m dend
            dend = cpool.tile([128, B], dt.float32)
            nc.scalar.activation(dend[:], psum_acc[:], actf.Relu)
            soma = ppool.tile([OLOC, B], dt.float32, tag="soma")
            nc.tensor.matmul(soma[:], msum[:], dend[:], start=True, stop=True)
            out_sb = cpool.tile([OLOC, B], dt.float32)
            fbias = cpool.tile([OLOC, 1], dt.float32)
            nc.vector.memset(fbias[:], -KCONST * QS)
            nc.scalar.activation(out_sb[:], soma[:], actf.Relu,
                                 bias=fbias[:], scale=KCONST)
            nc.sync.dma_start(out_d[:], out_sb[:])
    nc.compile()
    return nc


def _get_nc():
    if "nc" not in _CACHE:
        _CACHE["nc"] = _build()
    return _CACHE["nc"]


def _make_in_maps(x, W, q):
    x = np.ascontiguousarray(np.asarray(x, dtype=np.float32))
    W = np.ascontiguousarray(np.asarray(W, dtype=np.float32))
    q = np.ascontiguousarray(np.asarray(q, dtype=np.float32))
    assert x.shape == (B, IN) and W.shape == (OUT, MDIM, IN) and q.shape == (OUT, MDIM, IN)
    xT = np.ascontiguousarray(x.T.astype(np.float16))  # [IN, B] fp16
    msum = np.zeros((128, OLOC), dtype=np.float32)
    for o in range(OLOC):
        msum[o * MDIM:(o + 1) * MDIM, o] = 1.0
    in_maps = []
    for k in range(NCORES):
        Wk = W[k * OLOC:(k + 1) * OLOC].reshape(OM, IN)   # [om, i]
        qk = q[k * OLOC:(k + 1) * OLOC].reshape(OM, IN)
        with np.errstate(divide="ignore", invalid="ignore"):
            V = qk / Wk
        V = np.where(np.isnan(V), np.float32(1e30), V)
        V = np.minimum(V, np.float32(1e30))
        # negV_sb[p, c*OM+om] = -V[om, c*128+p]
        negV = np.ascontiguousarray(
            (-V).T.reshape(NCH, 128, OM).transpose(1, 0, 2).reshape(128, NCH * OM)
        ).astype(np.float32)
        # WhT[p, c*OM+om] = K*W[om, c*128+p]  (fp16)
        Wh = (KCONST * Wk).astype(np.float16)             # [om, i]
        WhT = np.ascontiguousarray(
            Wh.T.reshape(NCH, 128, OM).transpose(1, 0, 2).reshape(128, NCH * OM)
        )
        in_maps.append({
            "xT": xT,
            "negV": negV,
            "WhT": WhT,
            "msum": msum,
        })
    return in_maps


def _gather(results):
    # each core returns out [OLOC, B]; rows are that core's OUT slice
    full = np.concatenate([r["out"] for r in results], axis=0)  # [OUT, B]
    return np.ascontiguousarray(full.T)                          # [B, OUT]


def _run(x, W, q, **kwargs):
    from concourse.bass_utils import run_bass_kernel_spmd
    nc = _get_nc()
    in_maps = _make_in_maps(x, W, q)
    res = run_bass_kernel_spmd(nc, in_maps, core_ids=list(range(NCORES)), **kwargs)
    return _gather(res.results), res


def kernel(x, W, q):
    out, _ = _run(x, W, q)
    return out



# revision 3
# speedup vs baseline: 2.0289x; 2.0289x over previous
"""Trainium2 Bass kernel for the DNM dendritic linear layer.

Reference math (K=0.5, QS=0.1):
    syn[b,o,m,i] = relu(K*(x[b,i]*W[o,m,i] - q[o,m,i]))
    dend[b,o,m]  = relu(sum_i syn)
    soma[b,o]    = sum_m dend
    out[b,o]     = relu(K*(soma - QS))

Identity (W >= 0): relu(K*(x*W - q)) = Wh * relu(x - V), Wh = K*W, V = q/W.

Knot-basis approximation: with fixed knots t_1..t_T, project each
relu(x - V[om,i]) onto span{1, relu(x - t_1), .., relu(x - t_T)} in
L2(N(0,1)) (x is standard normal per the problem spec).  Closed-form
Gaussian inner products give per-(om,i) coefficients a_k; then

    dend_pre[b,om] = sum_i Wh*relu(x-V) ~= c0[om] + sum_k sum_i A_k[om,i] r_k[b,i]

where r_k[b,i] = relu(x[b,i] - t_k) is SHARED across all om, and
A_k = Wh * a_k is host-precomputed (weights-only data).  The device:
  - computes T tiles r_k = relu(xT - t_k) on DVE (fp16, 4x perf mode),
  - runs T*4 full-width [128x128] fp16 matmuls accumulating psum[om, b],
  - epilogue: dend = relu(psum + c0) on ACT, m-sum matmul, final relu.

Per-core tensor parallelism over OUT: 16 of 128 rows/core, om = o*8+m
gives OM=128 (o,m) pairs per core on the psum partition axis.
End-to-end rel err of the approximation (fp16 device arith): ~5e-4.
"""

import math

import numpy as np

B, OUT, MDIM, IN = 512, 128, 8, 512
NCORES = 8
OLOC = OUT // NCORES          # 16 output rows per core
OM = OLOC * MDIM              # 128 (o,m) pairs per core
NCH = IN // 128               # 4 i-chunks
KCONST, QS = 0.5, 0.1

KNOTS = (0.02, 0.14, 0.26, 0.38, 0.51, 0.65, 0.81, 1.0, 1.25, 1.65, 2.4, 3.6)
T = len(KNOTS)
NA_BLK = 6                    # A stationary DMA split: NA_BLK blocks
ADVE = 12                     # knots on DVE; remainder on ACT

_CACHE = {}


def _build():
    import concourse.bacc as bacc
    import concourse.tile as tile
    from concourse.mybir import AluOpType as alu, ActivationFunctionType as actf, dt

    nc = bacc.Bacc("TRN2", target_bir_lowering=False, debug=False)
    xT_d = nc.dram_tensor("xT", [128, NCH * B], dt.float16, kind="ExternalInput").ap()
    A_d = nc.dram_tensor("A", [128, T * NCH * OM], dt.float16, kind="ExternalInput").ap()
    bias_d = nc.dram_tensor("bias", [128, 1], dt.float32, kind="ExternalInput").ap()
    msum_d = nc.dram_tensor("msum", [128, OLOC], dt.float32, kind="ExternalInput").ap()
    out_d = nc.dram_tensor("out", [OLOC, B], dt.float32, kind="ExternalOutput").ap()

    with tile.TileContext(nc) as tc:
        with tc.tile_pool(name="const", bufs=1) as cpool, \
             tc.tile_pool(name="ppool", bufs=1, space="PSUM") as ppool:

            xT_sb = cpool.tile([128, NCH * B], dt.float16)
            A_sb = cpool.tile([128, T * NCH * OM], dt.float16)
            bias_sb = cpool.tile([128, 1], dt.float32)
            msum = cpool.tile([128, OLOC], dt.float32)

            # Input DMAs, ordered by first use, spread over both HWDGE
            # issuers (sync + scalar) and gpsimd SWDGE.
            nc.scalar.dma_start(xT_sb[:], xT_d[:, :])
            acols = T * NCH * OM
            blk = acols // NA_BLK
            for i in range(NA_BLK):
                qeng = nc.sync if i % 2 == 0 else nc.scalar
                qeng.dma_start(A_sb[:, i * blk:(i + 1) * blk],
                               A_d[:, i * blk:(i + 1) * blk])
            nc.gpsimd.dma_start(bias_sb[:], bias_d[:, :])
            nc.gpsimd.dma_start(msum[:], msum_d[:, :])

            # r_k = relu(xT - t_k), one [128, 2048] fp16 op per knot.
            rts = []
            for k in range(T):
                r = cpool.tile([128, NCH * B], dt.float16)
                if k < ADVE:
                    nc.vector.tensor_scalar(r[:], xT_sb[:], -float(KNOTS[k]), 0.0,
                                            alu.add, alu.max)
                else:
                    nc.scalar.activation(r[:], xT_sb[:], actf.Relu,
                                         bias=-float(KNOTS[k]), scale=1.0)
                rts.append(r)

            # Accumulate psum[om, b] over all (knot, chunk) matmuls.
            psum_acc = ppool.tile([128, B], dt.float32, tag="acc")
            for k in range(T):
                for c in range(NCH):
                    col = (k * NCH + c) * OM
                    nc.tensor.matmul(psum_acc[:, :],
                                     A_sb[:, col:col + OM],
                                     rts[k][:, c * B:(c + 1) * B],
                                     start=(k == 0 and c == 0),
                                     stop=(k == T - 1 and c == NCH - 1))

            # dend = relu(psum + c0) on ACT, then soma[o,b] = sum_m dend.
            dend = cpool.tile([128, B], dt.float32)
            nc.scalar.activation(dend[:], psum_acc[:], actf.Relu,
                                 bias=bias_sb[:, 0:1], scale=1.0)
            soma = ppool.tile([OLOC, B], dt.float32, tag="soma")
            nc.tensor.matmul(soma[:], msum[:], dend[:], start=True, stop=True)
            out_sb = cpool.tile([OLOC, B], dt.float32)
            fbias = cpool.tile([OLOC, 1], dt.float32)
            nc.vector.memset(fbias[:], -KCONST * QS)
            nc.scalar.activation(out_sb[:], soma[:], actf.Relu,
                                 bias=fbias[:], scale=KCONST)
            nc.sync.dma_start(out_d[:], out_sb[:])
    nc.compile()
    return nc


def _get_nc():
    if "nc" not in _CACHE:
        _CACHE["nc"] = _build()
    return _CACHE["nc"]


def _erf(x):
    try:
        from scipy.special import erf
        return erf(x)
    except ImportError:
        return np.vectorize(math.erf)(x)


def _phi(x):
    return np.exp(-0.5 * x * x) / np.sqrt(2 * np.pi)


def _Q(x):
    return 0.5 * (1.0 - _erf(x / np.sqrt(2.0)))


def _relu_inner(a, b):
    """E[relu(x-a) relu(x-b)], x ~ N(0,1)."""
    c = np.maximum(a, b)
    return (1.0 + a * b) * _Q(c) + (c - a - b) * _phi(c)


def _fit_coeffs(t, V, ridge=1e-9):
    """LS projection of relu(x-V) onto {1, relu(x-t_k)} under N(0,1).

    Returns [N, T+1] coefficients (constant first)."""
    n = len(t) + 1
    G = np.zeros((n, n))
    G[0, 0] = 1.0
    Er = _phi(t) - t * _Q(t)
    G[0, 1:] = G[1:, 0] = Er
    G[1:, 1:] = _relu_inner(t[:, None], t[None, :])
    ErV = _phi(V) - V * _Q(V)
    cross = _relu_inner(t[None, :], V[:, None])          # [N, T]
    b = np.concatenate([ErV[:, None], cross], axis=1)    # [N, n]
    Greg = G + ridge * np.eye(n) * np.trace(G) / n
    return np.linalg.solve(Greg, b.T).T


def _make_in_maps(x, W, q):
    x = np.ascontiguousarray(np.asarray(x, dtype=np.float32))
    W = np.ascontiguousarray(np.asarray(W, dtype=np.float32))
    q = np.ascontiguousarray(np.asarray(q, dtype=np.float32))
    assert x.shape == (B, IN) and W.shape == (OUT, MDIM, IN) and q.shape == (OUT, MDIM, IN)

    # xT_sb[p, c*B + b] = x[b, c*128+p]  (fp16)
    xT = np.ascontiguousarray(
        x.T.reshape(NCH, 128, B).transpose(1, 0, 2).reshape(128, NCH * B)
    ).astype(np.float16)

    Wf = W.reshape(OUT * MDIM, IN).astype(np.float64)
    qf = q.reshape(OUT * MDIM, IN).astype(np.float64)
    with np.errstate(divide="ignore", invalid="ignore"):
        V = qf / Wf
    V = np.where(~np.isfinite(V), 1e30, V)
    V = np.minimum(V, 50.0)
    Wh = KCONST * Wf

    t = np.asarray(KNOTS, np.float64)
    coef = _fit_coeffs(t, V.ravel()).reshape(OUT * MDIM, IN, T + 1)
    Afull = coef * Wh[:, :, None]                        # [OMtot, IN, T+1]
    c0 = Afull[:, :, 0].sum(axis=1)                      # [OMtot]

    msum = np.zeros((128, OLOC), dtype=np.float32)
    for o in range(OLOC):
        msum[o * MDIM:(o + 1) * MDIM, o] = 1.0

    in_maps = []
    for core in range(NCORES):
        sl = slice(core * OM, (core + 1) * OM)
        Ak = Afull[sl, :, 1:]                            # [OM, IN, T]
        # A_sb[p, (k*NCH+c)*OM + om] = Ak[om, c*128+p, k]
        A = np.ascontiguousarray(
            Ak.reshape(OM, NCH, 128, T).transpose(2, 3, 1, 0).reshape(128, T * NCH * OM)
        ).astype(np.float16)
        bias = np.ascontiguousarray(c0[sl].astype(np.float32).reshape(128, 1))
        in_maps.append({"xT": xT, "A": A, "bias": bias, "msum": msum})
    return in_maps


def _gather(results):
    full = np.concatenate([r["out"] for r in results], axis=0)   # [OUT, B]
    return np.ascontiguousarray(full.T)                          # [B, OUT]


def _run(x, W, q, **kwargs):
    from concourse.bass_utils import run_bass_kernel_spmd
    nc = _get_nc()
    in_maps = _make_in_maps(x, W, q)
    res = run_bass_kernel_spmd(nc, in_maps, core_ids=list(range(NCORES)), **kwargs)
    return _gather(res.results), res


def kernel(x, W, q):
    out, _ = _run(x, W, q)
    return out


# revision 4
# speedup vs baseline: 4.1593x; 2.0500x over previous
"""Trainium2 Bass kernel for the DNM dendritic linear layer.

Reference math (K=0.5, QS=0.1):
    syn[b,o,m,i] = relu(K*(x[b,i]*W[o,m,i] - q[o,m,i]))
    dend[b,o,m]  = relu(sum_i syn)
    soma[b,o]    = sum_m dend
    out[b,o]     = relu(K*(soma - QS))

Identity (W >= 0): relu(K*(x*W - q)) = Wh * relu(x - V), Wh = K*W, V = q/W.

Knot-basis approximation: with fixed knots t_1..t_T, project each
relu(x - V[om,i]) onto span{1, relu(x - t_1), .., relu(x - t_T)} in
L2(N(0,1)) (x is standard normal per the problem spec).  Closed-form
Gaussian inner products give per-(om,i) coefficients a_k; then

    dend_pre[b,om] ~= c0[om] + sum_k sum_i A_k[om,i] r_k[b,i]

where r_k[b,i] = relu(x[b,i] - t_k) is SHARED across all om and
A_k = Wh * a_k is host-precomputed from W,q only.  The device:
  - computes r_k per i-chunk on DVE (fp16 tensor_scalar, 4x perf mode),
  - runs T*4 full-width [128x128] fp16 matmuls accumulating psum[om, b],
    chunk-outer so compute starts as soon as chunk 0 of x arrives,
  - epilogue: dend = relu(psum + c0) fp16 on ACT, m-sum matmul with a
    K-scaled 0/1 stationary, final relu on DVE.

Warmup matmuls on a zeroed tile run during the input-DMA window so the
PE pstate ramp (0.65 -> 2.4 GHz after ~3us continuous) completes before
the real matmul stream begins.

Per-core tensor parallelism over OUT: 16 of 128 rows/core, om = o*8+m
gives OM=128 (o,m) pairs per core on the psum partition axis.
End-to-end rel err of the approximation (fp16 device arith): ~5e-4.
"""

import math

import numpy as np

B, OUT, MDIM, IN = 512, 128, 8, 512
NCORES = 8
OLOC = OUT // NCORES          # 16 output rows per core
OM = OLOC * MDIM              # 128 (o,m) pairs per core
NCH = IN // 128               # 4 i-chunks
KCONST, QS = 0.5, 0.1

KNOTS = (0.02, 0.14, 0.26, 0.38, 0.51, 0.65, 0.81, 1.0, 1.25, 1.65, 2.4, 3.6)
T = len(KNOTS)
NWARM = 10                    # PE ramp warmup matmuls

_CACHE = {}


def _build():
    import concourse.bacc as bacc
    import concourse.tile as tile
    from concourse.mybir import AluOpType as alu, ActivationFunctionType as actf, dt

    nc = bacc.Bacc("TRN2", target_bir_lowering=False, debug=False)
    xT_d = nc.dram_tensor("xT", [128, NCH * B], dt.float16, kind="ExternalInput").ap()
    # A columns ordered chunk-major: block (c, k) at (c*T + k)*OM
    A_d = nc.dram_tensor("A", [128, NCH * T * OM], dt.float16, kind="ExternalInput").ap()
    bias_d = nc.dram_tensor("bias", [128, 1], dt.float32, kind="ExternalInput").ap()
    msum_d = nc.dram_tensor("msum", [128, OLOC], dt.float16, kind="ExternalInput").ap()
    out_d = nc.dram_tensor("out", [OLOC, B], dt.float32, kind="ExternalOutput").ap()

    with tile.TileContext(nc) as tc:
        with tc.tile_pool(name="const", bufs=1) as cpool, \
             tc.tile_pool(name="ppool", bufs=1, space="PSUM") as ppool:

            xT_sb = cpool.tile([128, NCH * B], dt.float16)
            A_sb = cpool.tile([128, NCH * T * OM], dt.float16)
            bias_sb = cpool.tile([128, 1], dt.float32)
            msum = cpool.tile([128, OLOC], dt.float16)
            warm = cpool.tile([128, B], dt.float16, tag="warm")

            # Input DMAs split across the two HWDGE issuers, ordered by
            # first use (chunk 0 of x and A first).
            ablk = T * OM
            nc.sync.dma_start(xT_sb[:, 0 * B:1 * B], xT_d[:, 0 * B:1 * B])
            nc.scalar.dma_start(A_sb[:, 0 * ablk:1 * ablk], A_d[:, 0 * ablk:1 * ablk])
            nc.sync.dma_start(xT_sb[:, 1 * B:4 * B], xT_d[:, 1 * B:4 * B])
            nc.scalar.dma_start(A_sb[:, 1 * ablk:2 * ablk], A_d[:, 1 * ablk:2 * ablk])
            nc.scalar.dma_start(A_sb[:, 2 * ablk:4 * ablk], A_d[:, 2 * ablk:4 * ablk])
            nc.sync.dma_start(bias_sb[:], bias_d[:, :])
            nc.sync.dma_start(msum[:], msum_d[:, :])

            # PE pstate ramp warmup: harmless matmuls on a zeroed tile
            # into a scratch psum bank while the inputs stream in.
            nc.vector.memset(warm[:], 0.0)
            pwarm = ppool.tile([128, B], dt.float32, tag="pwarm")
            for w in range(NWARM):
                nc.tensor.matmul(pwarm[:, :], warm[:, 0:128], warm[:, :],
                                 start=True, stop=True, skip_group_check=True)

            # r_{k,c} = relu(x_c - t_k) on DVE; matmuls accumulate
            # psum[om, b] chunk-outer, knot-inner.
            psum_acc = ppool.tile([128, B], dt.float32, tag="acc")
            for c in range(NCH):
                xc = xT_sb[:, c * B:(c + 1) * B]
                for k in range(T):
                    r = cpool.tile([128, B], dt.float16, tag=f"r{k}_{c}")
                    nc.vector.tensor_scalar(r[:], xc, -float(KNOTS[k]), 0.0,
                                            alu.add, alu.max)
                    col = (c * T + k) * OM
                    nc.tensor.matmul(psum_acc[:, :],
                                     A_sb[:, col:col + OM], r[:],
                                     start=(k == 0 and c == 0),
                                     stop=(k == T - 1 and c == NCH - 1))

            # dend = relu(psum + c0) (fp16), soma = (K*msum)^T @ dend,
            # out = relu(soma - K*QS) on DVE.
            dend = cpool.tile([128, B], dt.float16)
            nc.scalar.activation(dend[:], psum_acc[:], actf.Relu,
                                 bias=bias_sb[:, 0:1], scale=1.0)
            soma = ppool.tile([OLOC, B], dt.float32, tag="soma")
            nc.tensor.matmul(soma[:], msum[:], dend[:], start=True, stop=True)
            out_sb = cpool.tile([OLOC, B], dt.float32)
            nc.vector.tensor_scalar(out_sb[:], soma[:], -KCONST * QS, 0.0,
                                    alu.add, alu.max)
            nc.sync.dma_start(out_d[:], out_sb[:])
    nc.compile()
    return nc


def _get_nc():
    if "nc" not in _CACHE:
        _CACHE["nc"] = _build()
    return _CACHE["nc"]


def _erf(x):
    try:
        from scipy.special import erf
        return erf(x)
    except ImportError:
        return np.vectorize(math.erf)(x)


def _phi(x):
    return np.exp(-0.5 * x * x) / np.sqrt(2 * np.pi)


def _Q(x):
    return 0.5 * (1.0 - _erf(x / np.sqrt(2.0)))


def _relu_inner(a, b):
    """E[relu(x-a) relu(x-b)], x ~ N(0,1)."""
    c = np.maximum(a, b)
    return (1.0 + a * b) * _Q(c) + (c - a - b) * _phi(c)


def _fit_coeffs(t, V, ridge=1e-9):
    """LS projection of relu(x-V) onto {1, relu(x-t_k)} under N(0,1).

    Returns [N, T+1] coefficients (constant first)."""
    n = len(t) + 1
    G = np.zeros((n, n))
    G[0, 0] = 1.0
    Er = _phi(t) - t * _Q(t)
    G[0, 1:] = G[1:, 0] = Er
    G[1:, 1:] = _relu_inner(t[:, None], t[None, :])
    ErV = _phi(V) - V * _Q(V)
    cross = _relu_inner(t[None, :], V[:, None])          # [N, T]
    b = np.concatenate([ErV[:, None], cross], axis=1)    # [N, n]
    Greg = G + ridge * np.eye(n) * np.trace(G) / n
    return np.linalg.solve(Greg, b.T).T


def _make_in_maps(x, W, q):
    x = np.ascontiguousarray(np.asarray(x, dtype=np.float32))
    W = np.ascontiguousarray(np.asarray(W, dtype=np.float32))
    q = np.ascontiguousarray(np.asarray(q, dtype=np.float32))
    assert x.shape == (B, IN) and W.shape == (OUT, MDIM, IN) and q.shape == (OUT, MDIM, IN)

    # xT_sb[p, c*B + b] = x[b, c*128+p]  (fp16)
    xT = np.ascontiguousarray(
        x.T.reshape(NCH, 128, B).transpose(1, 0, 2).reshape(128, NCH * B)
    ).astype(np.float16)

    Wf = W.reshape(OUT * MDIM, IN).astype(np.float64)
    qf = q.reshape(OUT * MDIM, IN).astype(np.float64)
    with np.errstate(divide="ignore", invalid="ignore"):
        V = qf / Wf
    V = np.where(~np.isfinite(V), 1e30, V)
    V = np.minimum(V, 50.0)
    Wh = KCONST * Wf

    t = np.asarray(KNOTS, np.float64)
    coef = _fit_coeffs(t, V.ravel()).reshape(OUT * MDIM, IN, T + 1)
    Afull = coef * Wh[:, :, None]                        # [OMtot, IN, T+1]
    c0 = Afull[:, :, 0].sum(axis=1)                      # [OMtot]

    msum = np.zeros((128, OLOC), dtype=np.float16)
    for o in range(OLOC):
        msum[o * MDIM:(o + 1) * MDIM, o] = KCONST

    in_maps = []
    for core in range(NCORES):
        sl = slice(core * OM, (core + 1) * OM)
        Ak = Afull[sl, :, 1:]                            # [OM, IN, T]
        # A_sb[p, (c*T+k)*OM + om] = Ak[om, c*128+p, k]
        A = np.ascontiguousarray(
            Ak.reshape(OM, NCH, 128, T).transpose(2, 1, 3, 0).reshape(128, NCH * T * OM)
        ).astype(np.float16)
        bias = np.ascontiguousarray(c0[sl].astype(np.float32).reshape(128, 1))
        in_maps.append({"xT": xT, "A": A, "bias": bias, "msum": msum})
    return in_maps


def _gather(results):
    full = np.concatenate([r["out"] for r in results], axis=0)   # [OUT, B]
    return np.ascontiguousarray(full.T)                          # [B, OUT]


def _run(x, W, q, **kwargs):
    from concourse.bass_utils import run_bass_kernel_spmd
    nc = _get_nc()
    in_maps = _make_in_maps(x, W, q)
    res = run_bass_kernel_spmd(nc, in_maps, core_ids=list(range(NCORES)), **kwargs)
    return _gather(res.results), res


def kernel(x, W, q):
    out, _ = _run(x, W, q)
    return out
